# revision 15
# baseline (speedup 1.0000x reference)
"""Trainium2 Bass kernel for sparse-attention MultiHeadAttention.

Strategy (8 NeuronCores, batch-parallel):
  - Core b handles batch element b end-to-end (B == n_cores == 8).
  - Host does index/layout prep only: transposes for matmul layouts,
    per-core edge partitioning (edges belong to core batch[src]),
    scatter-index computation, and a query-axis permutation that places
    core b's graph nodes first so the edge-bias window is [0, n_cap).
  - Device per core:
      qhT/khT = (Wq q_b^T), (Wk k_b^T)  as (HID x N) "head-transposed"
      vh      = v_b Wv^T                as (N x HID) (key-partition rows)
      ew      = edge_attr_aug @ WeT_aug (per-edge per-head bias values)
      scatter ew into a DRAM table at rows dst*n_cap + i_local (one
      indirect DMA; duplicate (src,dst) pairs pre-summed on device via a
      selection-matrix matmul within their chunk)
      scoresT[j,i] = sum_d khT[d,j] qhT[d,i]   (PE, per head, j-tiled)
      scoresT += edge bias (table readback) + (-1e9)*maskT  (DVE)
      attnT = exp(scoresT)  (ACT, no max-subtraction: inputs are tiny)
      out_augT[(d|1), i] += vh_aug[j, (d|1)]^T attnT[j, i]  (PE; the
      appended ones-column yields the softmax row sums for free)
      outT = out_augT[0:32] / rowsum (broadcast via DRAM roundtrip)
      y = outT_all^T stacked -> final Wo projection -> (N x HID) out
  - Host inverse-permutes rows and stacks core outputs.
"""

import math

import numpy as np

B, N, HID, H, EF = 8, 1024, 256, 8, 16
D = HID // H  # 32
P = 128
NT = N // P  # 8 j-tiles / n-tiles
SCALE = float(D) ** -0.5
NEG = -1.0e9

_PROGRAM_CACHE: dict = {}


def _build_program(n_cap: int, ncu_jt, ncd_jt):
    import concourse.bacc as bacc
    import concourse.bass as bass
    import concourse.mybir as mybir
    import concourse.tile as tile

    f32 = mybir.dt.float32
    i32 = mybir.dt.int32
    AF = mybir.ActivationFunctionType
    ALU = mybir.AluOpType

    NC = sum(ncu_jt) + sum(ncd_jt)  # total 128-edge chunks
    NE = NC * P
    # chunk -> (j-tile, is_dup)
    chunk_jt = []
    chunk_dup = []
    for jt in range(NT):
        chunk_jt += [jt] * (ncu_jt[jt] + ncd_jt[jt])
        chunk_dup += [False] * ncu_jt[jt] + [True] * ncd_jt[jt]
    TCOLS = (n_cap + 1) * 8  # per-partition floats in a (zero-padded) table
    assert n_cap <= 512

    nc = bacc.Bacc(
        "TRN2",
        target_bir_lowering=False,
        debug=False,
        enable_asserts=False,
    )
    dp = nc.declare_dram_parameter
    qT = dp("qT", [HID, N], f32, isOutput=False)
    kT = dp("kT", [HID, N], f32, isOutput=False)
    vT = dp("vT", [HID, N], f32, isOutput=False)
    maskF = dp("maskF", [N, N], f32, isOutput=False)
    WqT = dp("WqT", [HID, HID], f32, isOutput=False)
    WkT = dp("WkT", [HID, HID], f32, isOutput=False)
    WvT = dp("WvT", [HID, HID], f32, isOutput=False)
    WoT = dp("WoT", [HID, HID], f32, isOutput=False)
    bq = dp("bq", [HID, 1], f32, isOutput=False)
    bk = dp("bk", [HID, 1], f32, isOutput=False)
    bv = dp("bv", [1, HID], f32, isOutput=False)
    bo = dp("bo", [1, HID], f32, isOutput=False)
    WeTa = dp("WeTa", [EF + 1, H], f32, isOutput=False)
    attrT = dp("attrT", [EF + 1, NE], f32, isOutput=False)
    idxT = dp("idxT", [P, NC], i32, isOutput=False)
    identI = dp("identI", [P, P], f32, isOutput=False)
    out = dp("out", [N, HID], f32, isOutput=True)

    with tile.TileContext(nc) as tc:
        with (
            tc.tile_pool(name="dram", bufs=1, space="DRAM") as dram,
            tc.tile_pool(name="dram_rs", bufs=2, space="DRAM") as dram_rs,
            tc.tile_pool(name="const", bufs=1) as cons,
            tc.tile_pool(name="pers", bufs=1) as pers,
        ):
            # ---------------- constants / weights ----------------
            ident = cons.tile([P, P], f32)
            nc.sync.dma_start(out=ident[:], in_=identI[:, :])
            wq_sb = [cons.tile([P, HID], f32, tag=f"wq{t}", name=f"wq{t}") for t in range(2)]
            wk_sb = [cons.tile([P, HID], f32, tag=f"wk{t}", name=f"wk{t}") for t in range(2)]
            wv_sb = [cons.tile([P, HID], f32, tag=f"wv{t}", name=f"wv{t}") for t in range(2)]
            wo_sb = [cons.tile([P, HID], f32, tag=f"wo{t}", name=f"wo{t}") for t in range(2)]
            for t in range(2):
                nc.sync.dma_start(out=wq_sb[t][:], in_=WqT[t * P : (t + 1) * P, :])
                nc.sync.dma_start(out=wk_sb[t][:], in_=WkT[t * P : (t + 1) * P, :])
                nc.sync.dma_start(out=wv_sb[t][:], in_=WvT[t * P : (t + 1) * P, :])
                nc.sync.dma_start(out=wo_sb[t][:], in_=WoT[t * P : (t + 1) * P, :])
            we_sb = cons.tile([EF + 1, H], f32)
            nc.sync.dma_start(out=we_sb[:], in_=WeTa[:, :])
            bq_sb = [cons.tile([P, 1], f32, tag=f"bq{t}", name=f"bqs{t}") for t in range(2)]
            bk_sb = [cons.tile([P, 1], f32, tag=f"bk{t}", name=f"bks{t}") for t in range(2)]
            for t in range(2):
                nc.sync.dma_start(out=bq_sb[t][:], in_=bq[t * P : (t + 1) * P, :])
                nc.sync.dma_start(out=bk_sb[t][:], in_=bk[t * P : (t + 1) * P, :])
            bv_bc = cons.tile([P, HID], f32)
            nc.sync.dma_start(out=bv_bc[:], in_=bv[0:1, :].to_broadcast((P, HID)))
            bo_bc = cons.tile([P, HID], f32)
            nc.sync.dma_start(out=bo_bc[:], in_=bo[0:1, :].to_broadcast((P, HID)))

            # ---------------- edge-bias tables (one per j-tile) ----------------
            tables = [
                dram.tile([P * (n_cap + 1), 8], f32, tag=f"tab{jt}", name=f"tab{jt}")
                for jt in range(NT)
            ]
            ztile = pers.tile([P, TCOLS], f32)
            nc.vector.memset(ztile[:], 0.0)
            for jt in range(NT):
                zv = tables[jt][:].rearrange("a b -> (a b)").rearrange(
                    "(p f) -> p f", f=TCOLS
                )
                nc.gpsimd.dma_start(out=zv, in_=ztile[:])

            idx_sb = pers.tile([P, NC], i32)
            nc.sync.dma_start(out=idx_sb[:], in_=idxT[:, :])
            ew_sb = pers.tile([P, 8 * NC], f32)

            with (
                tc.tile_pool(name="attr", bufs=3) as attrp,
                tc.tile_pool(name="ewp", bufs=2, space="PSUM") as ewp,
                tc.tile_pool(name="dupp", bufs=2, space="PSUM") as dupp,
                tc.tile_pool(name="dups", bufs=3) as dups,
            ):
                APC = 16  # chunks per attr piece
                n_pieces = (NC + APC - 1) // APC
                for pc in range(n_pieces):
                    c0, c1 = pc * APC, min((pc + 1) * APC, NC)
                    at = attrp.tile([EF + 1, APC * P], f32, tag="attr")
                    nc.gpsimd.dma_start(
                        out=at[:, : (c1 - c0) * P], in_=attrT[:, c0 * P : c1 * P]
                    )
                    eps = ewp.tile([P, (c1 - c0) * 8], f32, tag="ewps")
                    for c in range(c0, c1):
                        lc = c - c0
                        nc.tensor.matmul(
                            out=eps[:, lc * 8 : lc * 8 + 8],
                            lhsT=at[:, lc * P : (lc + 1) * P],
                            rhs=we_sb[:],
                            start=True,
                            stop=True,
                        )
                    if not any(chunk_dup[c0:c1]):
                        # unique chunks: bulk copy PSUM -> ew_sb
                        nc.scalar.activation(
                            out=ew_sb[:, c0 * 8 : c1 * 8], in_=eps[:], func=AF.Copy
                        )
                    else:
                        for c in range(c0, c1):
                            lc = c - c0
                            if not chunk_dup[c]:
                                nc.scalar.activation(
                                    out=ew_sb[:, c * 8 : c * 8 + 8],
                                    in_=eps[:, lc * 8 : lc * 8 + 8],
                                    func=AF.Copy,
                                )
                                continue
                            # duplicate-group chunk: pre-sum rows sharing an
                            # index with a selection-matrix matmul
                            ewt = dups.tile([P, 8], f32, tag="ewt")
                            nc.scalar.activation(
                                out=ewt[:], in_=eps[:, lc * 8 : lc * 8 + 8], func=AF.Copy
                            )
                            idf = dups.tile([P, 1], f32, tag="idf")
                            nc.vector.tensor_copy(out=idf[:], in_=idx_sb[:, c : c + 1])
                            tp = dupp.tile([P, P], f32, tag="tp")
                            nc.tensor.transpose(
                                out=tp[:], in_=idf[:].to_broadcast((P, P)), identity=ident[:]
                            )
                            idft = dups.tile([P, P], f32, tag="idft")
                            nc.scalar.activation(out=idft[:], in_=tp[:], func=AF.Copy)
                            sel = dups.tile([P, P], f32, tag="sel")
                            nc.vector.tensor_tensor(
                                out=sel[:],
                                in0=idf[:].to_broadcast((P, P)),
                                in1=idft[:],
                                op=ALU.is_equal,
                            )
                            sp = dupp.tile([P, 8], f32, tag="sp")
                            nc.tensor.matmul(
                                out=sp[:], lhsT=sel[:], rhs=ewt[:], start=True, stop=True
                            )
                            nc.scalar.activation(
                                out=ew_sb[:, c * 8 : c * 8 + 8], in_=sp[:], func=AF.Copy
                            )

            # scatter per 128-edge chunk (HW indirect DMA requires a
            # single-column offset table; fused multi-chunk scatters
            # mis-execute on hardware). Chunks target their j-tile's table
            # so the 8 WAW chains run in parallel and each table readback
            # only waits for its own chunks.
            for c in range(NC):
                nc.gpsimd.indirect_dma_start(
                    out=tables[chunk_jt[c]][:],
                    out_offset=bass.IndirectOffsetOnAxis(ap=idx_sb[:, c : c + 1], axis=0),
                    in_=ew_sb[:, c * 8 : (c + 1) * 8],
                    in_offset=None,
                )

            # ---------------- projections ----------------
            qhT = [pers.tile([64, N], f32, tag=f"qhT{t}", name=f"qhT{t}") for t in range(4)]
            khT = [pers.tile([64, N], f32, tag=f"khT{t}", name=f"khT{t}") for t in range(4)]
            vha = [pers.tile([P, 33 * H], f32, tag=f"vha{j}", name=f"vha{j}") for j in range(NT)]

            with (
                tc.tile_pool(name="xin", bufs=2) as xin,
                tc.tile_pool(name="projp", bufs=2, space="PSUM") as projp,
            ):
                q_sb = [xin.tile([P, N], f32, tag=f"q{t}", name=f"qsb{t}") for t in range(2)]
                k_sb = [xin.tile([P, N], f32, tag=f"k{t}", name=f"ksb{t}") for t in range(2)]
                v_sb = [xin.tile([P, N], f32, tag=f"v{t}", name=f"vsb{t}") for t in range(2)]
                for t in range(2):
                    nc.sync.dma_start(out=q_sb[t][:], in_=qT[t * P : (t + 1) * P, :])
                    nc.sync.dma_start(out=k_sb[t][:], in_=kT[t * P : (t + 1) * P, :])
                    nc.sync.dma_start(out=v_sb[t][:], in_=vT[t * P : (t + 1) * P, :])

                for mt in range(2):
                    for ih in range(2):
                        ps = projp.tile([P, 512], f32, tag="prq")
                        for kt in range(2):
                            nc.tensor.matmul(
                                out=ps[:],
                                lhsT=wq_sb[kt][:, mt * P : (mt + 1) * P],
                                rhs=q_sb[kt][:, ih * 512 : (ih + 1) * 512],
                                start=(kt == 0),
                                stop=(kt == 1),
                            )
                        for half in range(2):
                            nc.scalar.activation(
                                out=qhT[2 * mt + half][:, ih * 512 : (ih + 1) * 512],
                                in_=ps[half * 64 : (half + 1) * 64, :],
                                func=AF.Identity,
                                bias=bq_sb[mt][half * 64 : (half + 1) * 64, :],
                                scale=SCALE,
                            )
                        ps2 = projp.tile([P, 512], f32, tag="prk")
                        for kt in range(2):
                            nc.tensor.matmul(
                                out=ps2[:],
                                lhsT=wk_sb[kt][:, mt * P : (mt + 1) * P],
                                rhs=k_sb[kt][:, ih * 512 : (ih + 1) * 512],
                                start=(kt == 0),
                                stop=(kt == 1),
                            )
                        for half in range(2):
                            nc.scalar.activation(
                                out=khT[2 * mt + half][:, ih * 512 : (ih + 1) * 512],
                                in_=ps2[half * 64 : (half + 1) * 64, :],
                                func=AF.Identity,
                                bias=bk_sb[mt][half * 64 : (half + 1) * 64, :],
                                scale=1.0,
                            )

                for jt in range(NT):
                    ps = projp.tile([P, HID], f32, tag="prv")
                    for kt in range(2):
                        nc.tensor.matmul(
                            out=ps[:],
                            lhsT=v_sb[kt][:, jt * P : (jt + 1) * P],
                            rhs=wv_sb[kt][:],
                            start=(kt == 0),
                            stop=(kt == 1),
                        )
                    for h in range(H):
                        nc.vector.tensor_tensor(
                            out=vha[jt][:, 33 * h : 33 * h + 32],
                            in0=ps[:, 32 * h : 32 * h + 32],
                            in1=bv_bc[:, 32 * h : 32 * h + 32],
                            op=ALU.add,
                        )
                    nc.vector.memset(vha[jt][:, 32::33], 1.0)

            # ---------------- mask + table readback ----------------
            mk = [pers.tile([P, N], f32, tag=f"mk{j}", name=f"mk{j}") for j in range(NT)]
            for jt in range(NT):
                nc.sync.dma_start(out=mk[jt][:], in_=maskF[jt * P : (jt + 1) * P, :])
            tb = [pers.tile([P, n_cap * 8], f32, tag=f"tb{j}", name=f"tbr{j}") for j in range(NT)]
            for jt in range(NT):
                nc.gpsimd.dma_start(
                    out=tb[jt][:],
                    in_=tables[jt][: P * n_cap, :]
                    .rearrange("a b -> (a b)")
                    .rearrange("(p f) -> p f", f=n_cap * 8),
                )

            # ---------------- attention ----------------
            oT = [pers.tile([P, N], f32, tag=f"oT{t}", name=f"oT{t}") for t in range(2)]
            with (
                tc.tile_pool(name="scp", bufs=2, space="PSUM") as scp,
                tc.tile_pool(name="oap", bufs=2, space="PSUM") as oap,
                tc.tile_pool(name="att", bufs=3) as att,
                tc.tile_pool(name="rsp", bufs=2) as rsp,
            ):
                for h in range(H):
                    ht, hr = h // 2, (h % 2) * 32
                    ot, orow = h // 4, (h % 4) * 32
                    oa = [oap.tile([33, 512], f32, tag=f"oa{i}", name=f"oa{i}_{h}") for i in range(2)]
                    for jt in range(NT):
                        sc = [scp.tile([P, 512], f32, tag=f"sc{i}", name=f"sc{i}_{h}_{jt}") for i in range(2)]
                        for ih in range(2):
                            nc.tensor.matmul(
                                out=sc[ih][:],
                                lhsT=khT[ht][hr : hr + 32, jt * P : (jt + 1) * P],
                                rhs=qhT[ht][hr : hr + 32, ih * 512 : (ih + 1) * 512],
                                start=True,
                                stop=True,
                            )
                        # edge bias lands in window [0, n_cap) of i-half 0
                        nc.vector.tensor_tensor(
                            out=sc[0][:, :n_cap],
                            in0=sc[0][:, :n_cap],
                            in1=tb[jt][:, h::8],
                            op=ALU.add,
                        )
                        at_t = att.tile([P, N], f32, tag="attn")
                        for ih in range(2):
                            nc.vector.tensor_tensor(
                                out=sc[ih][:],
                                in0=sc[ih][:],
                                in1=mk[jt][:, ih * 512 : (ih + 1) * 512],
                                op=ALU.add,
                            )
                            nc.scalar.activation(
                                out=at_t[:, ih * 512 : (ih + 1) * 512],
                                in_=sc[ih][:],
                                func=AF.Exp,
                            )
                        for ih in range(2):
                            nc.tensor.matmul(
                                out=oa[ih][:],
                                lhsT=vha[jt][:, 33 * h : 33 * h + 33],
                                rhs=at_t[:, ih * 512 : (ih + 1) * 512],
                                start=(jt == 0),
                                stop=(jt == NT - 1),
                            )
                    # normalize: divide by rowsum (row 32), bcast via DRAM
                    rs = rsp.tile([1, N], f32, tag="rs")
                    for ih in range(2):
                        nc.scalar.activation(
                            out=rs[:, ih * 512 : (ih + 1) * 512],
                            in_=oa[ih][32:33, :],
                            func=AF.Copy,
                        )
                    rcp = rsp.tile([1, N], f32, tag="rcp")
                    nc.vector.reciprocal(out=rcp[:], in_=rs[:])
                    rs_d = dram_rs.tile([1, N], f32, tag="rsd")
                    nc.gpsimd.dma_start(out=rs_d[:], in_=rcp[:])
                    rb = rsp.tile([32, N], f32, tag="rb")
                    nc.gpsimd.dma_start(out=rb[:], in_=rs_d[0:1, :].to_broadcast((32, N)))
                    for ih in range(2):
                        nc.vector.tensor_tensor(
                            out=oT[ot][orow : orow + 32, ih * 512 : (ih + 1) * 512],
                            in0=oa[ih][0:32, :],
                            in1=rb[:, ih * 512 : (ih + 1) * 512],
                            op=ALU.mult,
                        )

            # ---------------- output projection ----------------
            with (
                tc.tile_pool(name="yp", bufs=4, space="PSUM") as yp,
                tc.tile_pool(name="ys", bufs=3) as ys,
            ):
                for nt in range(NT):
                    py = yp.tile([P, HID], f32, tag="py")
                    for ct in range(2):
                        nc.tensor.matmul(
                            out=py[:],
                            lhsT=oT[ct][:, nt * P : (nt + 1) * P],
                            rhs=wo_sb[ct][:],
                            start=(ct == 0),
                            stop=(ct == 1),
                        )
                    y_sb = ys.tile([P, HID], f32, tag="y")
                    nc.vector.tensor_tensor(
                        out=y_sb[:], in0=py[:], in1=bo_bc[:], op=ALU.add
                    )
                    nc.gpsimd.dma_start(out=out[nt * P : (nt + 1) * P, :], in_=y_sb[:])

    nc.compile()
    return nc


def _prep_edges(src, dst, starts, n_cap, b):
    """Per-core, per-j-tile edge chunking.

    Returns list of NT entries: (uniq_eids, uniq_keys, dup_chunks) where
    dup_chunks is a list of (eids, keys) per dup chunk; eids index into
    this core's edge arrays; keys are table-local rows
    (dst % P) * n_cap + i_local.
    """
    il = src - starts[b]
    jt_of = dst // P
    key = ((dst % P) * n_cap + il).astype(np.int64)
    out = []
    for jt in range(NT):
        sel = np.flatnonzero(jt_of == jt)
        keys = key[sel]
        order = np.argsort(keys, kind="stable")
        sel, keys = sel[order], keys[order]
        uq, inv, cnt = np.unique(keys, return_inverse=True, return_counts=True)
        is_single = (cnt == 1)[inv]
        s_eid, s_key = sel[is_single], keys[is_single]
        d_eid, d_key = sel[~is_single], keys[~is_single]
        dup_chunks = []
        if len(d_eid):
            bounds = np.flatnonzero(np.diff(d_key)) + 1
            gs_list = np.concatenate([[0], bounds])
            ge_list = np.concatenate([bounds, [len(d_key)]])
            cur_e, cur_k, used = [], [], 0
            for gs, ge in zip(gs_list, ge_list):
                g = ge - gs
                if used + g > P:
                    dup_chunks.append((np.concatenate(cur_e), np.concatenate(cur_k)))
                    cur_e, cur_k, used = [], [], 0
                cur_e.append(d_eid[gs:ge])
                cur_k.append(d_key[gs:ge])
                used += g
            if used:
                dup_chunks.append((np.concatenate(cur_e), np.concatenate(cur_k)))
        out.append((s_eid, s_key, dup_chunks))
    return out


def _prepare(inputs):
    q = np.ascontiguousarray(np.asarray(inputs["q"], np.float32))
    k = np.ascontiguousarray(np.asarray(inputs["k"], np.float32))
    v = np.ascontiguousarray(np.asarray(inputs["v"], np.float32))
    edge_attr = np.ascontiguousarray(np.asarray(inputs["edge_attr"], np.float32))
    edge_index = np.asarray(inputs["edge_index"]).astype(np.int64)
    batch = np.asarray(inputs["batch"]).astype(np.int64)
    attn_mask = np.asarray(inputs["attn_mask"]).astype(bool)
    Wq = np.asarray(inputs["Wq"], np.float32)
    Wk = np.asarray(inputs["Wk"], np.float32)
    Wv = np.asarray(inputs["Wv"], np.float32)
    We = np.asarray(inputs["We"], np.float32)
    Wo = np.asarray(inputs["Wo"], np.float32)
    bq = np.asarray(inputs["bq"], np.float32)
    bk = np.asarray(inputs["bk"], np.float32)
    bv = np.asarray(inputs["bv"], np.float32)
    be = np.asarray(inputs["be"], np.float32)
    bo = np.asarray(inputs["bo"], np.float32)

    counts = np.bincount(batch, minlength=B)
    starts = np.concatenate([[0], np.cumsum(counts)[:-1]]).astype(np.int64)
    n_cap = max(int(counts.max()), 8)

    src, dst = edge_index[0], edge_index[1]
    gid = batch[src]

    # per-core edge slots, chunked per j-tile
    per_core = []
    for b in range(B):
        m = np.flatnonzero(gid == b)
        per_core.append((m, _prep_edges(src[m], dst[m], starts, n_cap, b)))
    # uniform per-jt chunk counts across cores
    ncu_jt = [0] * NT
    ncd_jt = [0] * NT
    for _, jts in per_core:
        for jt in range(NT):
            s_eid, _, dups = jts[jt]
            ncu_jt[jt] = max(ncu_jt[jt], (len(s_eid) + P - 1) // P)
            ncd_jt[jt] = max(ncd_jt[jt], len(dups))
    ncu_jt = [max(c, 1) for c in ncu_jt]
    NC = sum(ncu_jt) + sum(ncd_jt)
    TRASH = np.int32(P * n_cap)

    key = (n_cap, tuple(ncu_jt), tuple(ncd_jt))
    if key in _PROGRAM_CACHE:
        nc = _PROGRAM_CACHE[key]
    else:
        nc = _build_program(n_cap, ncu_jt, ncd_jt)
        _PROGRAM_CACHE[key] = nc

    in_maps = []
    perms = []
    for b in range(B):
        m, jts = per_core[b]
        slot_e = np.full(NC * P, -1, np.int64)
        slot_k = np.full(NC * P, -1, np.int64)
        pos = 0
        for jt in range(NT):
            s_eid, s_key, dups = jts[jt]
            ns = len(s_eid)
            slot_e[pos * P : pos * P + ns] = s_eid
            slot_k[pos * P : pos * P + ns] = s_key
            pos += ncu_jt[jt]
            for ce, ck in dups:
                slot_e[pos * P : pos * P + len(ce)] = ce
                slot_k[pos * P : pos * P + len(ck)] = ck
                pos += 1
            pos += ncd_jt[jt] - len(dups)
        assert pos == NC
        valid = slot_e >= 0
        attrT_h = np.zeros((EF + 1, NC * P), np.float32)
        if valid.any():
            rows = m[slot_e[valid]]
            attrT_h[:EF, valid] = edge_attr[rows].T
            attrT_h[EF, valid] = 1.0
        idx_full = np.full(NC * P, TRASH, np.int32)
        idx_full[valid] = slot_k[valid].astype(np.int32)
        idxT_h = np.ascontiguousarray(idx_full.reshape(NC, P).T)

        s_b, n_b = int(starts[b]), int(counts[b])
        perm = np.concatenate(
            [np.arange(s_b, s_b + n_b), np.arange(0, s_b), np.arange(s_b + n_b, N)]
        )
        perms.append(perm)

        maskF_h = np.where(attn_mask[b].T[:, perm], np.float32(NEG), np.float32(0.0))

        in_maps.append(
            {
                "qT": np.ascontiguousarray(q[b].T[:, perm]),
                "kT": np.ascontiguousarray(k[b].T),
                "vT": np.ascontiguousarray(v[b].T),
                "maskF": np.ascontiguousarray(maskF_h),
                "WqT": np.ascontiguousarray(Wq.T),
                "WkT": np.ascontiguousarray(Wk.T),
                "WvT": np.ascontiguousarray(Wv.T),
                "WoT": np.ascontiguousarray(Wo.T),
                "bq": bq.reshape(HID, 1).copy(),
                "bk": bk.reshape(HID, 1).copy(),
                "bv": bv.reshape(1, HID).copy(),
                "bo": bo.reshape(1, HID).copy(),
                "WeTa": np.ascontiguousarray(
                    np.concatenate([We.T, be.reshape(1, H)], axis=0)
                ),
                "attrT": attrT_h,
                "idxT": idxT_h,
                "identI": np.eye(P, dtype=np.float32),
            }
        )

    return nc, in_maps, perms


def kernel(_trace=False, **inputs):
    nc, in_maps, perms = _prepare(inputs)
    from concourse.bass_utils import run_bass_kernel_spmd

    res = run_bass_kernel_spmd(
        nc, in_maps, core_ids=list(range(B)), trace=_trace
    )
    outs = []
    for b in range(B):
        y = res.results[b]["out"]
        inv = np.empty(N, np.int64)
        inv[perms[b]] = np.arange(N)
        outs.append(y[inv])
    final = np.stack(outs).astype(np.float32)
    if _trace:
        kernel._last_results = res
    return final


# revision 19
# speedup vs baseline: 1.1605x; 1.1605x over previous
"""Trainium2 Bass kernel for sparse-attention MultiHeadAttention.

Strategy (8 NeuronCores, batch-parallel):
  - Core b handles batch element b end-to-end (B == n_cores == 8).
  - Host does index/layout prep only: transposes for matmul layouts,
    per-core edge partitioning (edges belong to core batch[src]),
    scatter-index computation, and a query-axis permutation that places
    core b's graph nodes first so the edge-bias window is [0, n_cap).
  - Device per core:
      qhT/khT = (Wq q_b^T), (Wk k_b^T)  as (HID x N) "head-transposed"
      vh      = v_b Wv^T                as (N x HID) (key-partition rows)
      ew      = edge_attr_aug @ WeT_aug (per-edge per-head bias values)
      scatter ew into a DRAM table at rows dst*n_cap + i_local (one
      indirect DMA; duplicate (src,dst) pairs pre-summed on device via a
      selection-matrix matmul within their chunk)
      scoresT[j,i] = sum_d khT[d,j] qhT[d,i]   (PE, per head, j-tiled)
      scoresT += edge bias (table readback) + (-1e9)*maskT  (DVE)
      attnT = exp(scoresT)  (ACT, no max-subtraction: inputs are tiny)
      out_augT[(d|1), i] += vh_aug[j, (d|1)]^T attnT[j, i]  (PE; the
      appended ones-column yields the softmax row sums for free)
      outT = out_augT[0:32] / rowsum (broadcast via DRAM roundtrip)
      y = outT_all^T stacked -> final Wo projection -> (N x HID) out
  - Host inverse-permutes rows and stacks core outputs.
"""

import math

import ml_dtypes
import numpy as np

BF16 = ml_dtypes.bfloat16

B, N, HID, H, EF = 8, 1024, 256, 8, 16
D = HID // H  # 32
P = 128
NT = N // P  # 8 j-tiles / n-tiles
SCALE = float(D) ** -0.5
NEG = -1.0e9

_PROGRAM_CACHE: dict = {}


def _build_program(n_cap: int, ncu_jt, ncd_jt):
    import concourse.bacc as bacc
    import concourse.bass as bass
    import concourse.mybir as mybir
    import concourse.tile as tile

    f32 = mybir.dt.float32
    f32r = mybir.dt.float32r
    bf16 = mybir.dt.bfloat16
    i32 = mybir.dt.int32

    def r(ap):
        return ap.bitcast(f32r)
    AF = mybir.ActivationFunctionType
    ALU = mybir.AluOpType

    NC = sum(ncu_jt) + sum(ncd_jt)  # total 128-edge chunks
    NE = NC * P
    # chunk -> (j-tile, is_dup)
    chunk_jt = []
    chunk_dup = []
    for jt in range(NT):
        chunk_jt += [jt] * (ncu_jt[jt] + ncd_jt[jt])
        chunk_dup += [False] * ncu_jt[jt] + [True] * ncd_jt[jt]
    TCOLS = (n_cap + 1) * 8  # per-partition floats in a (zero-padded) table
    assert n_cap <= 512

    nc = bacc.Bacc(
        "TRN2",
        target_bir_lowering=False,
        debug=False,
        enable_asserts=False,
    )
    dp = nc.declare_dram_parameter
    qT = dp("qT", [HID, N], f32, isOutput=False)
    kT = dp("kT", [HID, N], f32, isOutput=False)
    vT = dp("vT", [HID, N], f32, isOutput=False)
    maskF = dp("maskF", [N, N], f32, isOutput=False)
    WqT = dp("WqT", [HID, HID], f32, isOutput=False)
    WkT = dp("WkT", [HID, HID], f32, isOutput=False)
    WvT = dp("WvT", [HID, HID], f32, isOutput=False)
    WoT = dp("WoT", [HID, HID], f32, isOutput=False)
    bq = dp("bq", [HID, 1], f32, isOutput=False)
    bk = dp("bk", [HID, 1], f32, isOutput=False)
    bv = dp("bv", [1, HID], f32, isOutput=False)
    bo = dp("bo", [1, HID], f32, isOutput=False)
    WeTa = dp("WeTa", [EF + 1, H], bf16, isOutput=False)
    attrT = dp("attrT", [EF + 1, NE], bf16, isOutput=False)
    idxT = dp("idxT", [P, NC], i32, isOutput=False)
    identI = dp("identI", [P, P], f32, isOutput=False)
    out = dp("out", [N, HID], f32, isOutput=True)

    with tile.TileContext(nc) as tc:
        with (
            tc.tile_pool(name="dram", bufs=1, space="DRAM") as dram,
            tc.tile_pool(name="dram_rs", bufs=2, space="DRAM") as dram_rs,
            tc.tile_pool(name="const", bufs=1) as cons,
            tc.tile_pool(name="pers", bufs=1) as pers,
        ):
            # ---------------- constants / weights ----------------
            ident = cons.tile([P, P], f32)
            nc.sync.dma_start(out=ident[:], in_=identI[:, :])
            wq_sb = [cons.tile([P, HID], f32, tag=f"wq{t}", name=f"wq{t}") for t in range(2)]
            wk_sb = [cons.tile([P, HID], f32, tag=f"wk{t}", name=f"wk{t}") for t in range(2)]
            wv_sb = [cons.tile([P, HID], f32, tag=f"wv{t}", name=f"wv{t}") for t in range(2)]
            wo_sb = [cons.tile([P, HID], f32, tag=f"wo{t}", name=f"wo{t}") for t in range(2)]
            for t in range(2):
                nc.sync.dma_start(out=wq_sb[t][:], in_=WqT[t * P : (t + 1) * P, :])
                nc.sync.dma_start(out=wk_sb[t][:], in_=WkT[t * P : (t + 1) * P, :])
                nc.sync.dma_start(out=wv_sb[t][:], in_=WvT[t * P : (t + 1) * P, :])
                nc.sync.dma_start(out=wo_sb[t][:], in_=WoT[t * P : (t + 1) * P, :])
            we_sb = cons.tile([EF + 1, H], bf16)
            nc.sync.dma_start(out=we_sb[:], in_=WeTa[:, :])
            bq_sb = [cons.tile([P, 1], f32, tag=f"bq{t}", name=f"bqs{t}") for t in range(2)]
            bk_sb = [cons.tile([P, 1], f32, tag=f"bk{t}", name=f"bks{t}") for t in range(2)]
            for t in range(2):
                nc.sync.dma_start(out=bq_sb[t][:], in_=bq[t * P : (t + 1) * P, :])
                nc.sync.dma_start(out=bk_sb[t][:], in_=bk[t * P : (t + 1) * P, :])
            bv_bc = cons.tile([P, HID], f32)
            nc.sync.dma_start(out=bv_bc[:], in_=bv[0:1, :].to_broadcast((P, HID)))
            bo_bc = cons.tile([P, HID], f32)
            nc.sync.dma_start(out=bo_bc[:], in_=bo[0:1, :].to_broadcast((P, HID)))

            # ---------------- edge-bias tables (one per j-tile) ----------------
            tables = [
                dram.tile([P * (n_cap + 1), 8], f32, tag=f"tab{jt}", name=f"tab{jt}")
                for jt in range(NT)
            ]
            ztile = pers.tile([P, TCOLS], f32)
            nc.vector.memset(ztile[:], 0.0)
            for jt in range(NT):
                zv = tables[jt][:].rearrange("a b -> (a b)").rearrange(
                    "(p f) -> p f", f=TCOLS
                )
                nc.gpsimd.dma_start(out=zv, in_=ztile[:])

            idx_sb = pers.tile([P, NC], i32)
            nc.sync.dma_start(out=idx_sb[:], in_=idxT[:, :])
            ew_sb = pers.tile([P, 8 * NC], f32)

            with (
                tc.tile_pool(name="attr", bufs=3) as attrp,
                tc.tile_pool(name="ewp", bufs=2, space="PSUM") as ewp,
                tc.tile_pool(name="dupp", bufs=2, space="PSUM") as dupp,
                tc.tile_pool(name="dups", bufs=3) as dups,
            ):
                APC = 16  # chunks per attr piece
                n_pieces = (NC + APC - 1) // APC
                for pc in range(n_pieces):
                    c0, c1 = pc * APC, min((pc + 1) * APC, NC)
                    at = attrp.tile([EF + 1, APC * P], bf16, tag="attr")
                    nc.gpsimd.dma_start(
                        out=at[:, : (c1 - c0) * P], in_=attrT[:, c0 * P : c1 * P]
                    )
                    eps = ewp.tile([P, (c1 - c0) * 8], f32, tag="ewps")
                    for c in range(c0, c1):
                        lc = c - c0
                        nc.tensor.matmul(
                            out=eps[:, lc * 8 : lc * 8 + 8],
                            lhsT=at[:, lc * P : (lc + 1) * P],
                            rhs=we_sb[:],
                            start=True,
                            stop=True,
                        )
                    if not any(chunk_dup[c0:c1]):
                        # unique chunks: bulk copy PSUM -> ew_sb
                        nc.scalar.activation(
                            out=ew_sb[:, c0 * 8 : c1 * 8], in_=eps[:], func=AF.Copy
                        )
                    else:
                        for c in range(c0, c1):
                            lc = c - c0
                            if not chunk_dup[c]:
                                nc.scalar.activation(
                                    out=ew_sb[:, c * 8 : c * 8 + 8],
                                    in_=eps[:, lc * 8 : lc * 8 + 8],
                                    func=AF.Copy,
                                )
                                continue
                            # duplicate-group chunk: pre-sum rows sharing an
                            # index with a selection-matrix matmul
                            ewt = dups.tile([P, 8], f32, tag="ewt")
                            nc.scalar.activation(
                                out=ewt[:], in_=eps[:, lc * 8 : lc * 8 + 8], func=AF.Copy
                            )
                            idf = dups.tile([P, 1], f32, tag="idf")
                            nc.vector.tensor_copy(out=idf[:], in_=idx_sb[:, c : c + 1])
                            tp = dupp.tile([P, P], f32, tag="tp")
                            nc.tensor.transpose(
                                out=tp[:], in_=idf[:].to_broadcast((P, P)), identity=ident[:]
                            )
                            idft = dups.tile([P, P], f32, tag="idft")
                            nc.scalar.activation(out=idft[:], in_=tp[:], func=AF.Copy)
                            sel = dups.tile([P, P], f32, tag="sel")
                            nc.vector.tensor_tensor(
                                out=sel[:],
                                in0=idf[:].to_broadcast((P, P)),
                                in1=idft[:],
                                op=ALU.is_equal,
                            )
                            sp = dupp.tile([P, 8], f32, tag="sp")
                            nc.tensor.matmul(
                                out=sp[:], lhsT=sel[:], rhs=ewt[:], start=True, stop=True
                            )
                            nc.scalar.activation(
                                out=ew_sb[:, c * 8 : c * 8 + 8], in_=sp[:], func=AF.Copy
                            )

            # scatter per 128-edge chunk (HW indirect DMA requires a
            # single-column offset table; fused multi-chunk scatters
            # mis-execute on hardware). Chunks target their j-tile's table
            # so the 8 WAW chains run in parallel and each table readback
            # only waits for its own chunks.
            for c in range(NC):
                nc.gpsimd.indirect_dma_start(
                    out=tables[chunk_jt[c]][:],
                    out_offset=bass.IndirectOffsetOnAxis(ap=idx_sb[:, c : c + 1], axis=0),
                    in_=ew_sb[:, c * 8 : (c + 1) * 8],
                    in_offset=None,
                )

            # ---------------- projections ----------------
            qhT = [pers.tile([64, N], f32, tag=f"qhT{t}", name=f"qhT{t}") for t in range(4)]
            khT = [pers.tile([64, N], f32, tag=f"khT{t}", name=f"khT{t}") for t in range(4)]
            vha = [pers.tile([P, 33 * H], f32, tag=f"vha{j}", name=f"vha{j}") for j in range(NT)]

            with (
                tc.tile_pool(name="xin", bufs=2) as xin,
                tc.tile_pool(name="projp", bufs=2, space="PSUM") as projp,
            ):
                q_sb = [xin.tile([P, N], f32, tag=f"q{t}", name=f"qsb{t}") for t in range(2)]
                k_sb = [xin.tile([P, N], f32, tag=f"k{t}", name=f"ksb{t}") for t in range(2)]
                v_sb = [xin.tile([P, N], f32, tag=f"v{t}", name=f"vsb{t}") for t in range(2)]
                for t in range(2):
                    nc.sync.dma_start(out=q_sb[t][:], in_=qT[t * P : (t + 1) * P, :])
                    nc.sync.dma_start(out=k_sb[t][:], in_=kT[t * P : (t + 1) * P, :])
                    nc.sync.dma_start(out=v_sb[t][:], in_=vT[t * P : (t + 1) * P, :])

                for mt in range(2):
                    for ih in range(2):
                        ps = projp.tile([P, 512], f32, tag="prq")
                        for kt in range(2):
                            nc.tensor.matmul(
                                out=ps[:],
                                lhsT=r(wq_sb[kt][:, mt * P : (mt + 1) * P]),
                                rhs=r(q_sb[kt][:, ih * 512 : (ih + 1) * 512]),
                                start=(kt == 0),
                                stop=(kt == 1),
                            )
                        for half in range(2):
                            nc.scalar.activation(
                                out=qhT[2 * mt + half][:, ih * 512 : (ih + 1) * 512],
                                in_=ps[half * 64 : (half + 1) * 64, :],
                                func=AF.Identity,
                                bias=bq_sb[mt][half * 64 : (half + 1) * 64, :],
                                scale=SCALE,
                            )
                        ps2 = projp.tile([P, 512], f32, tag="prk")
                        for kt in range(2):
                            nc.tensor.matmul(
                                out=ps2[:],
                                lhsT=r(wk_sb[kt][:, mt * P : (mt + 1) * P]),
                                rhs=r(k_sb[kt][:, ih * 512 : (ih + 1) * 512]),
                                start=(kt == 0),
                                stop=(kt == 1),
                            )
                        for half in range(2):
                            nc.scalar.activation(
                                out=khT[2 * mt + half][:, ih * 512 : (ih + 1) * 512],
                                in_=ps2[half * 64 : (half + 1) * 64, :],
                                func=AF.Identity,
                                bias=bk_sb[mt][half * 64 : (half + 1) * 64, :],
                                scale=1.0,
                            )

                for jt in range(NT):
                    ps = projp.tile([P, HID], f32, tag="prv")
                    for kt in range(2):
                        nc.tensor.matmul(
                            out=ps[:],
                            lhsT=r(v_sb[kt][:, jt * P : (jt + 1) * P]),
                            rhs=r(wv_sb[kt][:]),
                            start=(kt == 0),
                            stop=(kt == 1),
                        )
                    for h in range(H):
                        nc.vector.tensor_tensor(
                            out=vha[jt][:, 33 * h : 33 * h + 32],
                            in0=ps[:, 32 * h : 32 * h + 32],
                            in1=bv_bc[:, 32 * h : 32 * h + 32],
                            op=ALU.add,
                        )
                    nc.vector.memset(vha[jt][:, 32::33], 1.0)

            # ---------------- mask + table readback ----------------
            mk = [pers.tile([P, N], f32, tag=f"mk{j}", name=f"mk{j}") for j in range(NT)]
            for jt in range(NT):
                nc.sync.dma_start(out=mk[jt][:], in_=maskF[jt * P : (jt + 1) * P, :])
            tb = [pers.tile([P, n_cap * 8], f32, tag=f"tb{j}", name=f"tbr{j}") for j in range(NT)]
            for jt in range(NT):
                nc.gpsimd.dma_start(
                    out=tb[jt][:],
                    in_=tables[jt][: P * n_cap, :]
                    .rearrange("a b -> (a b)")
                    .rearrange("(p f) -> p f", f=n_cap * 8),
                )

            # ---------------- attention ----------------
            # 4 groups of 2 heads; j-tile inner so each (group, jt) visit
            # only needs table jt -> attention streams behind the 8
            # per-table scatter chains.
            oT = [pers.tile([P, N], f32, tag=f"oT{t}", name=f"oT{t}") for t in range(2)]
            with (
                tc.tile_pool(name="scp", bufs=2, space="PSUM") as scp,
                tc.tile_pool(name="oap", bufs=1, space="PSUM") as oap,
                tc.tile_pool(name="att", bufs=3) as att,
                tc.tile_pool(name="rsp", bufs=2) as rsp,
            ):
                for hg in range(H // 2):
                    heads = [2 * hg, 2 * hg + 1]
                    oa = {}
                    for hi in range(2):
                        for ih in range(2):
                            oa[(hi, ih)] = oap.tile(
                                [33, 512], f32, tag=f"oa{hi}{ih}", name=f"oa{hg}_{hi}{ih}"
                            )
                    for jt in range(NT):
                        for hi, h in enumerate(heads):
                            ht, hr = h // 2, (h % 2) * 32
                            sc = [
                                scp.tile([P, 512], f32, tag=f"sc{i}", name=f"sc{i}_{h}_{jt}")
                                for i in range(2)
                            ]
                            for ih in range(2):
                                nc.tensor.matmul(
                                    out=sc[ih][:],
                                    lhsT=r(khT[ht][hr : hr + 32, jt * P : (jt + 1) * P]),
                                    rhs=r(qhT[ht][hr : hr + 32, ih * 512 : (ih + 1) * 512]),
                                    start=True,
                                    stop=True,
                                )
                            # edge bias lands in window [0, n_cap) of i-half 0
                            nc.vector.tensor_tensor(
                                out=sc[0][:, :n_cap],
                                in0=sc[0][:, :n_cap],
                                in1=tb[jt][:, h::8],
                                op=ALU.add,
                            )
                            at_t = att.tile([P, N], f32, tag="attn", name=f"at_{h}_{jt}")
                            for ih in range(2):
                                nc.vector.tensor_tensor(
                                    out=sc[ih][:],
                                    in0=sc[ih][:],
                                    in1=mk[jt][:, ih * 512 : (ih + 1) * 512],
                                    op=ALU.add,
                                )
                                nc.scalar.activation(
                                    out=at_t[:, ih * 512 : (ih + 1) * 512],
                                    in_=sc[ih][:],
                                    func=AF.Exp,
                                )
                            for ih in range(2):
                                nc.tensor.matmul(
                                    out=oa[(hi, ih)][:],
                                    lhsT=r(vha[jt][:, 33 * h : 33 * h + 33]),
                                    rhs=r(at_t[:, ih * 512 : (ih + 1) * 512]),
                                    start=(jt == 0),
                                    stop=(jt == NT - 1),
                                )
                    # normalize: multiply by 1/rowsum (row 32), bcast via DRAM
                    for hi, h in enumerate(heads):
                        ot, orow = h // 4, (h % 4) * 32
                        rs = rsp.tile([1, N], f32, tag="rs", name=f"rs{h}")
                        for ih in range(2):
                            nc.scalar.activation(
                                out=rs[:, ih * 512 : (ih + 1) * 512],
                                in_=oa[(hi, ih)][32:33, :],
                                func=AF.Copy,
                            )
                        rcp = rsp.tile([1, N], f32, tag="rcp", name=f"rcp{h}")
                        nc.vector.reciprocal(out=rcp[:], in_=rs[:])
                        rs_d = dram_rs.tile([1, N], f32, tag="rsd", name=f"rsd{h}")
                        nc.gpsimd.dma_start(out=rs_d[:], in_=rcp[:])
                        rb = rsp.tile([32, N], f32, tag="rb", name=f"rb{h}")
                        nc.gpsimd.dma_start(
                            out=rb[:], in_=rs_d[0:1, :].to_broadcast((32, N))
                        )
                        for ih in range(2):
                            nc.vector.tensor_tensor(
                                out=oT[ot][orow : orow + 32, ih * 512 : (ih + 1) * 512],
                                in0=oa[(hi, ih)][0:32, :],
                                in1=rb[:, ih * 512 : (ih + 1) * 512],
                                op=ALU.mult,
                            )

            # ---------------- output projection ----------------
            with (
                tc.tile_pool(name="yp", bufs=4, space="PSUM") as yp,
                tc.tile_pool(name="ys", bufs=3) as ys,
            ):
                for nt in range(NT):
                    py = yp.tile([P, HID], f32, tag="py")
                    for ct in range(2):
                        nc.tensor.matmul(
                            out=py[:],
                            lhsT=r(oT[ct][:, nt * P : (nt + 1) * P]),
                            rhs=r(wo_sb[ct][:]),
                            start=(ct == 0),
                            stop=(ct == 1),
                        )
                    y_sb = ys.tile([P, HID], f32, tag="y")
                    nc.vector.tensor_tensor(
                        out=y_sb[:], in0=py[:], in1=bo_bc[:], op=ALU.add
                    )
                    nc.gpsimd.dma_start(out=out[nt * P : (nt + 1) * P, :], in_=y_sb[:])

    nc.compile()
    return nc


def _prep_edges(src, dst, starts, n_cap, b):
    """Per-core, per-j-tile edge chunking.

    Returns list of NT entries: (uniq_eids, uniq_keys, dup_chunks) where
    dup_chunks is a list of (eids, keys) per dup chunk; eids index into
    this core's edge arrays; keys are table-local rows
    (dst % P) * n_cap + i_local.
    """
    il = src - starts[b]
    jt_of = dst // P
    key = ((dst % P) * n_cap + il).astype(np.int64)
    out = []
    for jt in range(NT):
        sel = np.flatnonzero(jt_of == jt)
        keys = key[sel]
        order = np.argsort(keys, kind="stable")
        sel, keys = sel[order], keys[order]
        uq, inv, cnt = np.unique(keys, return_inverse=True, return_counts=True)
        is_single = (cnt == 1)[inv]
        s_eid, s_key = sel[is_single], keys[is_single]
        d_eid, d_key = sel[~is_single], keys[~is_single]
        dup_chunks = []
        if len(d_eid):
            bounds = np.flatnonzero(np.diff(d_key)) + 1
            gs_list = np.concatenate([[0], bounds])
            ge_list = np.concatenate([bounds, [len(d_key)]])
            cur_e, cur_k, used = [], [], 0
            for gs, ge in zip(gs_list, ge_list):
                g = ge - gs
                if used + g > P:
                    dup_chunks.append((np.concatenate(cur_e), np.concatenate(cur_k)))
                    cur_e, cur_k, used = [], [], 0
                cur_e.append(d_eid[gs:ge])
                cur_k.append(d_key[gs:ge])
                used += g
            if used:
                dup_chunks.append((np.concatenate(cur_e), np.concatenate(cur_k)))
        out.append((s_eid, s_key, dup_chunks))
    return out


def _prepare(inputs):
    q = np.ascontiguousarray(np.asarray(inputs["q"], np.float32))
    k = np.ascontiguousarray(np.asarray(inputs["k"], np.float32))
    v = np.ascontiguousarray(np.asarray(inputs["v"], np.float32))
    edge_attr = np.ascontiguousarray(np.asarray(inputs["edge_attr"], np.float32))
    edge_index = np.asarray(inputs["edge_index"]).astype(np.int64)
    batch = np.asarray(inputs["batch"]).astype(np.int64)
    attn_mask = np.asarray(inputs["attn_mask"]).astype(bool)
    Wq = np.asarray(inputs["Wq"], np.float32)
    Wk = np.asarray(inputs["Wk"], np.float32)
    Wv = np.asarray(inputs["Wv"], np.float32)
    We = np.asarray(inputs["We"], np.float32)
    Wo = np.asarray(inputs["Wo"], np.float32)
    bq = np.asarray(inputs["bq"], np.float32)
    bk = np.asarray(inputs["bk"], np.float32)
    bv = np.asarray(inputs["bv"], np.float32)
    be = np.asarray(inputs["be"], np.float32)
    bo = np.asarray(inputs["bo"], np.float32)

    counts = np.bincount(batch, minlength=B)
    starts = np.concatenate([[0], np.cumsum(counts)[:-1]]).astype(np.int64)
    n_cap = max(int(counts.max()), 8)

    src, dst = edge_index[0], edge_index[1]
    gid = batch[src]

    # per-core edge slots, chunked per j-tile
    per_core = []
    for b in range(B):
        m = np.flatnonzero(gid == b)
        per_core.append((m, _prep_edges(src[m], dst[m], starts, n_cap, b)))
    # uniform per-jt chunk counts across cores
    ncu_jt = [0] * NT
    ncd_jt = [0] * NT
    for _, jts in per_core:
        for jt in range(NT):
            s_eid, _, dups = jts[jt]
            ncu_jt[jt] = max(ncu_jt[jt], (len(s_eid) + P - 1) // P)
            ncd_jt[jt] = max(ncd_jt[jt], len(dups))
    ncu_jt = [max(c, 1) for c in ncu_jt]
    NC = sum(ncu_jt) + sum(ncd_jt)
    TRASH = np.int32(P * n_cap)

    key = (n_cap, tuple(ncu_jt), tuple(ncd_jt))
    if key in _PROGRAM_CACHE:
        nc = _PROGRAM_CACHE[key]
    else:
        nc = _build_program(n_cap, ncu_jt, ncd_jt)
        _PROGRAM_CACHE[key] = nc

    in_maps = []
    perms = []
    for b in range(B):
        m, jts = per_core[b]
        slot_e = np.full(NC * P, -1, np.int64)
        slot_k = np.full(NC * P, -1, np.int64)
        pos = 0
        for jt in range(NT):
            s_eid, s_key, dups = jts[jt]
            ns = len(s_eid)
            slot_e[pos * P : pos * P + ns] = s_eid
            slot_k[pos * P : pos * P + ns] = s_key
            pos += ncu_jt[jt]
            for ce, ck in dups:
                slot_e[pos * P : pos * P + len(ce)] = ce
                slot_k[pos * P : pos * P + len(ck)] = ck
                pos += 1
            pos += ncd_jt[jt] - len(dups)
        assert pos == NC
        valid = slot_e >= 0
        attrT_h = np.zeros((EF + 1, NC * P), BF16)
        if valid.any():
            rows = m[slot_e[valid]]
            attrT_h[:EF, valid] = edge_attr[rows].T.astype(BF16)
            attrT_h[EF, valid] = 1.0
        idx_full = np.full(NC * P, TRASH, np.int32)
        idx_full[valid] = slot_k[valid].astype(np.int32)
        idxT_h = np.ascontiguousarray(idx_full.reshape(NC, P).T)

        s_b, n_b = int(starts[b]), int(counts[b])
        perm = np.concatenate(
            [np.arange(s_b, s_b + n_b), np.arange(0, s_b), np.arange(s_b + n_b, N)]
        )
        perms.append(perm)

        maskF_h = np.where(attn_mask[b].T[:, perm], np.float32(NEG), np.float32(0.0))

        in_maps.append(
            {
                "qT": np.ascontiguousarray(q[b].T[:, perm]),
                "kT": np.ascontiguousarray(k[b].T),
                "vT": np.ascontiguousarray(v[b].T),
                "maskF": np.ascontiguousarray(maskF_h),
                "WqT": np.ascontiguousarray(Wq.T),
                "WkT": np.ascontiguousarray(Wk.T),
                "WvT": np.ascontiguousarray(Wv.T),
                "WoT": np.ascontiguousarray(Wo.T),
                "bq": bq.reshape(HID, 1).copy(),
                "bk": bk.reshape(HID, 1).copy(),
                "bv": bv.reshape(1, HID).copy(),
                "bo": bo.reshape(1, HID).copy(),
                "WeTa": np.ascontiguousarray(
                    np.concatenate([We.T, be.reshape(1, H)], axis=0).astype(BF16)
                ),
                "attrT": attrT_h,
                "idxT": idxT_h,
                "identI": np.eye(P, dtype=np.float32),
            }
        )

    return nc, in_maps, perms


def kernel(_trace=False, **inputs):
    nc, in_maps, perms = _prepare(inputs)
    from concourse.bass_utils import run_bass_kernel_spmd

    res = run_bass_kernel_spmd(
        nc, in_maps, core_ids=list(range(B)), trace=_trace
    )
    outs = []
    for b in range(B):
        y = res.results[b]["out"]
        inv = np.empty(N, np.int64)
        inv[perms[b]] = np.arange(N)
        outs.append(y[inv])
    final = np.stack(outs).astype(np.float32)
    if _trace:
        kernel._last_results = res
    return final
